# revision 17
# baseline (speedup 1.0000x reference)
"""Trainium2 Bass kernel for MultiHeadedAttentionSANM.

Per-core (data-parallel over batch, 8 cores, B=1 each):
  - x is host-cast to bf16; x^T is built with PE transposes (bf16, 1 cyc/row
    — half the cost of the f32 path).
  - qkv^T = (x @ Wqkv)^T on PE (bf16): q^T over all tokens; k^T and a second
    v^T only for the ~50% of tokens with mask=1, gathered host-side into a
    compact xc (TK tokens). v^T over all tokens in f32 (FSMN needs them).
  - FSMN: depthwise conv over time in (d, t) layout on DVE (bf16 taps, f32
    residual), in place on v^T; mask-muls on GPSIMD.
  - attention: scores computed transposed (compact keys on partitions) so the
    exp output feeds the ctx matmul directly as the rhs stream; masked/padded
    keys get a -30000 exp bias. Softmax denominator Z via a ones-weight PE
    pass; normalization delayed to SBUF (per-head PSUM + 1/Z broadcast).
"""

import os
import sys

for _p in ("/opt/trn_rl_repo", "/root/.axon_site/_ro/trn_rl_repo"):
    if os.path.isdir(_p) and _p not in sys.path:
        sys.path.append(_p)

from contextlib import ExitStack

import numpy as np

import concourse.bass as bass
import concourse.mybir as mybir
import concourse.tile as tile
from concourse import bacc
from concourse import bass_utils
from concourse.masks import make_identity

T, D, H, DK, KS, PAD = 2048, 512, 4, 128, 11, 5
NCORES = 8
NT = T // 128          # 16 t-blocks of 128
NC = D // 128          # 4 d-chunks of 128
SCALE = float(DK) ** -0.5
MASK_NEG = -30000.0

F32 = mybir.dt.float32
BF16 = mybir.dt.bfloat16
AF = mybir.ActivationFunctionType
OP = mybir.AluOpType

ATT_DT = os.environ.get("SANM_ATT_DT", "bf16")   # bf16 | f32
REPS = int(os.environ.get("SANM_REPS", "1"))     # timing: repeat body in one NEFF
FSMN_DT = os.environ.get("SANM_FSMN_DT", "bf16")  # f32 | bf16


def _bcast_vec(ap, nrows):
    """Broadcast a flat [N] DRAM AP across partitions -> [nrows, N]."""
    return bass.AP(tensor=ap.tensor, offset=ap.offset, ap=[[0, nrows]] + list(ap.ap))


def _tiles(total, step=512):
    out, p = [], 0
    while p < total:
        n = min(step, total - p)
        rem = total - p - n
        if 0 < rem < 256:
            n = (n + rem) // 2
            n = (n + 127) // 128 * 128
        out.append((p, n))
        p += n
    return out


def build_kernel_body(tc, aps, TK, rep=0):
    nc = tc.nc
    (xb_d, mask_d, xcb_d, cbias_d, wqkv_d, bqkv_d, wout_d, bout_d, fw_d,
     out_d) = aps
    R = f"r{rep}_" if rep else ""
    TKC = TK // 128  # compact key chunks

    att_store = F32 if ATT_DT == "f32" else BF16

    stack = ExitStack()
    consts = stack.enter_context(tc.tile_pool(name=R + "consts", bufs=1))
    work = stack.enter_context(tc.tile_pool(name=R + "work", bufs=2))
    ps = stack.enter_context(tc.tile_pool(name=R + "ps", bufs=1, space="PSUM"))
    dram = stack.enter_context(tc.tile_pool(name=R + "dram", bufs=2, space="DRAM"))

    # p_main holds all long-lived tensors (whole kernel); p_x nests inside it
    # (LIFO) and is released after the qkv matmuls to reclaim x^T space.
    main_cm = tc.tile_pool(name=R + "p_main", bufs=1)
    x_cm = tc.tile_pool(name=R + "p_x", bufs=1)
    p_main = main_cm.__enter__()
    p_x = x_cm.__enter__()

    # ---------------- constants ----------------
    ident = consts.tile([128, 128], F32, name="ident", tag="ident")
    make_identity(nc, ident)
    ident_b = consts.tile([128, 128], att_store, name="ident_b", tag="ident_b")
    nc.vector.tensor_copy(ident_b, ident)

    ones_att = consts.tile([128, 1], att_store, name="ones_att", tag="ones_att")
    nc.vector.memset(ones_att, 1.0)

    # compact-key exp bias (0 valid / -30000 padded), as columns (128, TKC)
    mbias = consts.tile([128, TKC], F32, name="mbias", tag="mbias")
    nc.sync.dma_start(out=mbias, in_=cbias_d.rearrange("(c p) -> p c", p=128))

    # mask broadcast across partitions (128, T) bf16 (exact for 0/1), for FSMN
    mrow = consts.tile([128, T], BF16, name="mrow", tag="mrow")
    nc.gpsimd.dma_start(out=mrow, in_=_bcast_vec(mask_d, 128))

    # biases as per-partition columns
    bq = consts.tile([128, 12], F32, name="bq", tag="bq")
    nc.sync.dma_start(out=bq, in_=bqkv_d.rearrange("(c p) -> p c", p=128))
    bo = consts.tile([128, NC], F32, name="bo", tag="bo")
    nc.sync.dma_start(out=bo, in_=bout_d.rearrange("(c p) -> p c", p=128))

    # fsmn weights (128, NC, KS); center tap += 1 (folds the residual)
    wadj = consts.tile([128, NC, KS], F32, name="wadj", tag="wadj")
    nc.scalar.dma_start(out=wadj, in_=fw_d.rearrange("(c p) o k -> p c (o k)", p=128))
    if FSMN_DT == "f32":
        nc.vector.tensor_scalar_add(
            wadj[:, :, PAD : PAD + 1], wadj[:, :, PAD : PAD + 1], 1.0
        )

    # Wout (128, NC, 512) in attention dtype, loaded directly (host-cast bf16)
    wo = consts.tile([128, NC, D], att_store, name="wo", tag="wo")
    nc.scalar.dma_start(
        out=wo, in_=wout_d.rearrange("(c p) f -> p c f", p=128)
    )

    # ---------------- x^T and xc^T (PE transposes, bf16 1cyc/row) ------------
    # (DMA xbar transpose races with concurrent regular DMA traffic on real
    # hardware — it silently corrupted 7/8 cores — so transpose on PE.)
    # Transposes for 4 consecutive t-blocks of one d-chunk accumulate in a
    # single [128, 512] PSUM tile so the PSUM->SBUF copy is one wide
    # instruction per group instead of four narrow ones (per-instruction
    # overhead on ACT/DVE is ~25% of a 512-wide op).
    def transpose_in(src_d, dsts, nti):
        for g0 in range(0, nti, 4):
            gn = min(4, nti - g0)
            tps = [
                ps.tile([128, 128 * gn], BF16, name=f"tp{c}", tag="s", bufs=4)
                for c in range(NC)
            ]
            for i in range(gn):
                ti = g0 + i
                xn = work.tile([128, D], BF16, name="xnat", tag="xnat", bufs=3)
                (nc.sync if ti % 2 == 0 else nc.scalar).dma_start(
                    out=xn, in_=src_d[ti * 128 : (ti + 1) * 128, :]
                )
                for c in range(NC):
                    nc.tensor.transpose(
                        tps[c][:, i * 128 : (i + 1) * 128],
                        xn[:, c * 128 : (c + 1) * 128],
                        ident_b,
                    )
            for c in range(NC):
                dst = dsts[c][:, g0 * 128 : (g0 + gn) * 128]
                if c % 2 == 0:
                    nc.vector.tensor_copy(dst, tps[c])
                else:
                    nc.scalar.copy(dst, tps[c])

    xT = [p_x.tile([128, T], BF16, name=f"xT{c}", tag=f"xT{c}") for c in range(NC)]
    transpose_in(xb_d, xT, NT)
    xcT = [
        p_x.tile([128, TK], BF16, name=f"xcT{c}", tag=f"xcT{c}") for c in range(NC)
    ]
    transpose_in(xcb_d, xcT, TKC)

    # ---------------- qkv^T = (x @ Wqkv)^T ----------------
    # q on full tokens; k only compact; v full (FSMN) and compact (attention)
    qT = [p_main.tile([128, T], att_store, name=f"qT{h}", tag=f"qT{h}") for h in range(H)]
    kTc = [p_main.tile([128, TK], att_store, name=f"kTc{h}", tag=f"kTc{h}") for h in range(H)]
    vT = [p_main.tile([128, T], F32, name=f"vT{c}", tag=f"vT{c}") for c in range(NC)]
    vcT = [p_main.tile([128, TK], att_store, name=f"vcT{c}", tag=f"vcT{c}") for c in range(NC)]

    def project(f, src, tspans, sink):
        """psum[128, n] = Wqkv[:, f-block].T @ src over d-chunks, then sink."""
        wqf = work.tile([128, NC, 128], BF16, name="wqf", tag="wqf", bufs=3)
        wqf_src = wqkv_d[:, f * 128 : (f + 1) * 128].rearrange("(c p) f -> p c f", p=128)
        (nc.scalar if f % 2 else nc.sync).dma_start(out=wqf, in_=wqf_src)
        for t0, n in tspans:
            mm = ps.tile([128, 512], F32, name="mmq", tag="s", bufs=4)
            for dc in range(NC):
                nc.tensor.matmul(
                    mm[:, :n],
                    wqf[:, dc, :],
                    src[dc][:, t0 : t0 + n],
                    start=(dc == 0),
                    stop=(dc == NC - 1),
                )
            sink(mm, t0, n)

    for f in range(4):  # q: full tokens -> qT (bf16), bias via ACT
        def sink_q(mm, t0, n, f=f):
            nc.scalar.activation(
                qT[f][:, t0 : t0 + n], mm[:, :n], AF.Identity,
                bias=bq[:, f : f + 1], scale=1.0,
            )
        project(f, xT, _tiles(T), sink_q)
    for f in range(4, 8):  # k: compact tokens -> kTc (bias via DVE; ACT is hot)
        def sink_k(mm, t0, n, f=f):
            nc.vector.tensor_scalar_add(
                kTc[f - 4][:, t0 : t0 + n], mm[:, :n], bq[:, f : f + 1]
            )
        project(f, xcT, _tiles(TK), sink_k)
    for f in range(8, 12):  # v full tokens (FSMN), f32, bias via DVE
        def sink_v(mm, t0, n, f=f):
            nc.vector.tensor_scalar_add(
                vT[f - 8][:, t0 : t0 + n], mm[:, :n], bq[:, f : f + 1]
            )
        project(f, xT, _tiles(T), sink_v)
    for f in range(8, 12):  # v compact tokens (attention), bias via DVE
        def sink_vc(mm, t0, n, f=f):
            nc.vector.tensor_scalar_add(
                vcT[f - 8][:, t0 : t0 + n], mm[:, :n], bq[:, f : f + 1]
            )
        project(f, xcT, _tiles(TK), sink_vc)
    x_cm.__exit__(None, None, None)  # frees xT, xcT

    # ------------- compact v natural (PE transposes of vcT, att dtype) --------
    vh = [
        p_main.tile([128, TKC, 128], att_store, name=f"vh{h}", tag=f"vh{h}")
        for h in range(H)
    ]
    for h in range(H):
        for g0 in range(0, TKC, 4):
            gn = min(4, TKC - g0)
            tpv = ps.tile([128, gn, 128], att_store, name="tpv", tag="s", bufs=4)
            for i in range(gn):
                jc = g0 + i
                nc.tensor.transpose(
                    tpv[:, i, :],
                    vcT[h][:, jc * 128 : (jc + 1) * 128],
                    ident_b,
                )
            nc.scalar.copy(vh[h][:, g0 : g0 + gn, :], tpv)

    # -------- FSMN (d, t layout), in place on vT; result lands back in vT -----
    for c in range(NC):
        # vm = v * m (in place)
        nc.gpsimd.tensor_tensor(vT[c], vT[c], mrow, op=OP.mult)
        if FSMN_DT == "f32":
            acc = p_main.tile([128, T], F32, name=f"facc{c}", tag="facc")
            # center tap first (w[5]+1 folds the residual), full width
            nc.any.tensor_scalar_mul(acc, vT[c], wadj[:, c, PAD : PAD + 1])
            src_t = vT[c]
        else:
            vmb = p_main.tile([128, T], BF16, name=f"vmb{c}", tag="fscratch")
            nc.any.tensor_copy(vmb, vT[c])
            acc = p_main.tile([128, T], BF16, name=f"facc{c}", tag="facc")
            nc.any.tensor_scalar_mul(acc, vmb, wadj[:, c, PAD : PAD + 1])
            src_t = vmb
        for k in list(range(0, PAD)) + list(range(PAD + 1, KS)):
            s = k - PAD
            lo, hi = max(0, -s), T - max(0, s)
            nc.vector.scalar_tensor_tensor(
                acc[:, lo:hi],
                src_t[:, lo + s : hi + s],
                wadj[:, c, k : k + 1],
                acc[:, lo:hi],
                OP.mult,
                OP.add,
            )
        if FSMN_DT == "f32":
            nc.gpsimd.tensor_tensor(acc, acc, mrow, op=OP.mult)
            # final masked conv + bias overwrites vT[c] (all tap reads are done)
            nc.any.tensor_scalar_add(vT[c], acc, bo[:, c : c + 1])
        else:
            # residual in f32: out = (conv_bf + vm) * m + bo; vm*m = vm
            facc2 = p_main.tile([128, T], F32, name=f"facc2{c}", tag="fscratch2")
            nc.vector.tensor_tensor(facc2, acc, mrow, op=OP.mult)
            nc.gpsimd.tensor_tensor(vT[c], facc2, vT[c], op=OP.add)
            nc.any.tensor_scalar_add(vT[c], vT[c], bo[:, c : c + 1])

    # ---------------- attention ----------------
    zd = dram.tile([H * T], F32, name="zd", tag="zd", bufs=1)
    ctxT = [
        p_main.tile([128, T], att_store, name=f"ctxT{h}", tag=f"ctxT{h}")
        for h in range(H)
    ]
    for h in range(H):
        for qs in range(4):  # query spans of 512
            isl = slice(qs * 512, (qs + 1) * 512)
            ctx_ps = ps.tile([128, 512], F32, name="ctx_ps", tag="actx", bufs=2)
            # sum of exp tiles over key chunks, accumulated on the (otherwise
            # idle) GPSIMD engine; the softmax denominator then needs only ONE
            # ones-matmul on PE per span instead of one per key chunk (the
            # ones-matmul uses 1/128 of the PE array — it was 20% of PE time).
            esum = work.tile([128, 512], att_store, name="esum", tag="esum", bufs=2)
            for jc in range(TKC):
                s_ps = ps.tile([128, 512], F32, name="s_ps", tag="s", bufs=4)
                nc.tensor.matmul(
                    s_ps,
                    kTc[h][:, jc * 128 : (jc + 1) * 128],
                    qT[h][:, isl],
                    start=True,
                    stop=True,
                    skip_group_check=True,
                )
                eT = work.tile([128, 512], att_store, name="eT", tag="eT", bufs=4)
                nc.scalar.activation(
                    eT, s_ps, AF.Exp, bias=mbias[:, jc : jc + 1], scale=SCALE
                )
                nc.tensor.matmul(
                    ctx_ps,
                    vh[h][:, jc, :],
                    eT,
                    start=(jc == 0),
                    stop=(jc == TKC - 1),
                    skip_group_check=True,
                )
                if jc == 0:
                    nc.gpsimd.tensor_copy(esum, eT)
                else:
                    nc.gpsimd.tensor_tensor(esum, esum, eT, op=OP.add)
            z_ps = ps.tile([1, 512], F32, name="z_ps", tag="z", bufs=2)
            nc.tensor.matmul(
                z_ps, ones_att, esum, start=True, stop=True, skip_group_check=True
            )
            rz = work.tile([1, 512], F32, name="rz", tag="rz", bufs=2)
            nc.vector.reciprocal(rz, z_ps)
            zsl = slice(h * T + qs * 512, h * T + (qs + 1) * 512)
            nc.sync.dma_start(out=zd[zsl], in_=rz)
            zb = work.tile([128, 512], BF16, name="zb", tag="zb", bufs=2)
            nc.gpsimd.dma_start(
                out=zb,
                in_=bass.AP(
                    tensor=zd.tensor,
                    offset=zd.offset + h * T + qs * 512,
                    ap=[[0, 128], [1, 512]],
                ),
            )
            nc.vector.tensor_tensor(ctxT[h][:, isl], ctx_ps, zb, op=OP.mult)

    # ---------------- out projection + fsmn add ----------------
    for tb in range(NT):
        op_ps = ps.tile([128, 512], F32, name="op_ps", tag="s", bufs=4)
        for h in range(H):
            nc.tensor.matmul(
                op_ps,
                ctxT[h][:, tb * 128 : (tb + 1) * 128],
                wo[:, h, :],
                start=(h == 0),
                stop=(h == H - 1),
            )
        # transpose this t-block of fsmn into natural layout
        ftp = ps.tile([128, 512], F32, name="ftp", tag="z", bufs=2)
        for c in range(NC):
            nc.tensor.transpose(
                ftp[:, c * 128 : (c + 1) * 128],
                vT[c][:, tb * 128 : (tb + 1) * 128],
                ident,
            )
        f_sb = work.tile([128, D], F32, name="f_sb", tag="f_sb", bufs=2)
        nc.scalar.copy(f_sb, ftp)
        o_sb = work.tile([128, D], F32, name="o_sb", tag="o_sb", bufs=2)
        nc.vector.tensor_tensor(o_sb, op_ps, f_sb, op=OP.add)
        nc.sync.dma_start(out=out_d[tb * 128 : (tb + 1) * 128, :], in_=o_sb)

    main_cm.__exit__(None, None, None)
    stack.close()


def _cached_spmd_fn(nc, donate=True):
    """Build (once) and cache a jitted SPMD executor for this nc.

    run_bass_kernel_spmd -> run_bass_via_pjrt re-creates the jax.jit closure
    on every call, so every kernel() invocation pays a full retrace + BIR
    serialization (scales with instruction count / REPS). Caching the jitted
    callable makes repeat calls pure dispatch+transfer+execute.

    donate=False builds a variant safe to call repeatedly on device-resident
    input buffers (for timing): the kernel writes every element of `out`, so
    uninitialized result buffers are fine.
    """
    cache = getattr(nc, "_sanm_exec", None)
    if cache is None:
        cache = nc._sanm_exec = {}
    ent = cache.get(donate)
    if ent is not None:
        return ent
    import jax
    from jax.experimental.shard_map import shard_map
    from jax.sharding import Mesh, PartitionSpec

    from concourse import bass2jax

    bass2jax.install_neuronx_cc_hook()
    assert nc.dbg_addr is None
    partition_name = nc.partition_id_tensor.name if nc.partition_id_tensor else None

    in_names, out_names, out_avals, zero_shapes = [], [], [], []
    for alloc in nc.m.functions[0].allocations:
        if not isinstance(alloc, mybir.MemoryLocationSet):
            continue
        name = alloc.memorylocations[0].name
        if alloc.kind == "ExternalInput":
            if name != partition_name:
                in_names.append(name)
        elif alloc.kind == "ExternalOutput":
            shape = tuple(alloc.tensor_shape)
            dtype = mybir.dt.np(alloc.dtype)
            out_avals.append(jax.core.ShapedArray(shape, dtype))
            out_names.append(name)
            zero_shapes.append((shape, dtype))
    n_params = len(in_names)
    n_outs = len(out_avals)
    all_names = list(in_names) + list(out_names)
    if partition_name is not None:
        all_names.append(partition_name)
    donate_idx = tuple(range(n_params, n_params + n_outs))

    def _body(*args):
        operands = list(args)
        if partition_name is not None:
            operands.append(bass2jax.partition_id_tensor())
        outs = bass2jax._bass_exec_p.bind(
            *operands,
            out_avals=tuple(out_avals),
            in_names=tuple(all_names),
            out_names=tuple(out_names),
            lowering_input_output_aliases=(),
            sim_require_finite=True,
            sim_require_nnan=True,
            nc=nc,
        )
        return tuple(outs)

    devices = jax.devices()[:NCORES]
    mesh = Mesh(np.asarray(devices), ("core",))
    sharded = jax.jit(
        shard_map(
            _body,
            mesh=mesh,
            in_specs=(PartitionSpec("core"),) * (n_params + n_outs),
            out_specs=(PartitionSpec("core"),) * n_outs,
            check_rep=False,
        ),
        donate_argnums=donate_idx if donate else (),
        keep_unused=True,
    )
    ent = (sharded, in_names, out_names, out_avals, zero_shapes, n_params, mesh)
    cache[donate] = ent
    return ent


def _run_cached(nc, in_maps):
    sharded, in_names, out_names, out_avals, zero_shapes, n_params, _ = _cached_spmd_fn(nc)
    concat_in = [
        np.concatenate([np.asarray(m[name]) for m in in_maps], axis=0)
        for name in in_names
    ]
    concat_zeros = [
        np.zeros((NCORES * s[0], *s[1:]), dt) for (s, dt) in zero_shapes
    ]
    out_arrs = sharded(*concat_in, *concat_zeros)
    return [
        {
            name: np.asarray(out_arrs[i]).reshape(NCORES, *out_avals[i].shape)[c]
            for i, name in enumerate(out_names)
        }
        for c in range(NCORES)
    ]


_CACHE = {}


def _build(TK):
    key = (ATT_DT, REPS, TK, FSMN_DT)
    if key in _CACHE:
        return _CACHE[key]
    nc = bacc.Bacc(
        "TRN2",
        target_bir_lowering=False,
        debug=False,
        enable_asserts=False,
        num_devices=NCORES,
    )
    aps = (
        nc.dram_tensor("xb", (T, D), BF16, kind="ExternalInput").ap(),
        nc.dram_tensor("mask", (T,), F32, kind="ExternalInput").ap(),
        nc.dram_tensor("xcb", (TK, D), BF16, kind="ExternalInput").ap(),
        nc.dram_tensor("cbias", (TK,), F32, kind="ExternalInput").ap(),
        nc.dram_tensor("Wqkv", (D, 3 * D), BF16, kind="ExternalInput").ap(),
        nc.dram_tensor("bqkv", (3 * D,), F32, kind="ExternalInput").ap(),
        nc.dram_tensor("Wout", (D, D), BF16, kind="ExternalInput").ap(),
        nc.dram_tensor("bout", (D,), F32, kind="ExternalInput").ap(),
        nc.dram_tensor("fsmn_w", (D, 1, KS), F32, kind="ExternalInput").ap(),
        nc.dram_tensor("out", (T, D), F32, kind="ExternalOutput").ap(),
    )
    with tile.TileContext(nc) as tc:
        for rep in range(REPS):
            build_kernel_body(tc, aps, TK, rep)
    nc.compile()
    _CACHE[key] = nc
    return nc


def _bf16():
    import ml_dtypes

    return ml_dtypes.bfloat16


def _compact(x_b, mask_b, TK):
    """Host-side gather of unmasked token rows, padded to TK (bf16)."""
    idx = np.nonzero(mask_b != 0)[0]
    n = len(idx)
    xc = np.zeros((TK, x_b.shape[1]), _bf16())
    xc[:n] = x_b[idx[:TK]].astype(_bf16())
    cb = np.full((TK,), MASK_NEG, np.float32)
    cb[:n] = 0.0
    return xc, cb


def _prep(x, mask, Wqkv, bqkv, Wout, bout, fsmn_w):
    """Full inputs -> (TK, per-core in_maps) with host-side bf16 casts."""
    bf16 = _bf16()
    x = np.ascontiguousarray(np.asarray(x, dtype=np.float32))
    mask = np.ascontiguousarray(np.asarray(mask, dtype=np.float32))
    Wqkv_b = np.ascontiguousarray(np.asarray(Wqkv, dtype=np.float32).astype(bf16))
    bqkv = np.ascontiguousarray(np.asarray(bqkv, dtype=np.float32))
    Wout_b = np.ascontiguousarray(np.asarray(Wout, dtype=np.float32).astype(bf16))
    bout = np.ascontiguousarray(np.asarray(bout, dtype=np.float32))
    fsmn_w = np.ascontiguousarray(np.asarray(fsmn_w, dtype=np.float32))

    counts = [int((mask[b, 0] != 0).sum()) for b in range(NCORES)]
    TK = min(T, max(256, int(-(-max(counts) // 128) * 128)))
    in_maps = []
    for b in range(NCORES):
        xc, cb = _compact(x[b], mask[b, 0], TK)
        in_maps.append(
            {
                "xb": np.ascontiguousarray(x[b].astype(bf16)),
                "mask": np.ascontiguousarray(mask[b, 0]),
                "xcb": xc,
                "cbias": cb,
                "Wqkv": Wqkv_b,
                "bqkv": bqkv,
                "Wout": Wout_b,
                "bout": bout,
                "fsmn_w": fsmn_w,
            }
        )
    return TK, in_maps


def kernel(x, mask, Wqkv, bqkv, Wout, bout, fsmn_w):
    TK, in_maps = _prep(x, mask, Wqkv, bqkv, Wout, bout, fsmn_w)
    nc = _build(TK)
    results = _run_cached(nc, in_maps)
    out = np.stack([results[b]["out"] for b in range(NCORES)], axis=0)
    return out


if __name__ == "__main__":
    rng = np.random.default_rng(0)
    ins = {
        "x": rng.standard_normal((NCORES, T, D), dtype=np.float32),
        "mask": rng.integers(0, 2, (NCORES, 1, T)).astype(np.float32),
        "Wqkv": (rng.standard_normal((D, 3 * D)) * 0.02).astype(np.float32),
        "bqkv": np.zeros((3 * D,), np.float32),
        "Wout": (rng.standard_normal((D, D)) * 0.02).astype(np.float32),
        "bout": np.zeros((D,), np.float32),
        "fsmn_w": (rng.standard_normal((D, 1, KS)) * 0.1).astype(np.float32),
    }
    out = kernel(**ins)
    print(out.shape, out.dtype, float(np.abs(out).max()))


# revision 22
# speedup vs baseline: 10.3530x; 10.3530x over previous
"""Trainium2 Bass kernel for MultiHeadedAttentionSANM.

Per-core (data-parallel over batch, 8 cores, B=1 each):
  - x is host-cast to bf16; x^T is built with PE transposes (bf16, 1 cyc/row
    — half the cost of the f32 path).
  - qkv^T = (x @ Wqkv)^T on PE (bf16): q^T over all tokens; k^T and a second
    v^T only for the ~50% of tokens with mask=1, gathered host-side into a
    compact xc (TK tokens). v^T over all tokens in f32 (FSMN needs them).
  - FSMN: depthwise conv over time in (d, t) layout on DVE (bf16 taps, f32
    residual), in place on v^T; mask-muls on GPSIMD.
  - attention: scores computed transposed (compact keys on partitions) so the
    exp output feeds the ctx matmul directly as the rhs stream; masked/padded
    keys get a -30000 exp bias. Softmax denominator Z via a ones-weight PE
    pass; normalization delayed to SBUF (per-head PSUM + 1/Z broadcast).
"""

import os
import sys

for _p in ("/opt/trn_rl_repo", "/root/.axon_site/_ro/trn_rl_repo"):
    if os.path.isdir(_p) and _p not in sys.path:
        sys.path.append(_p)

from contextlib import ExitStack

import numpy as np

import concourse.bass as bass
import concourse.mybir as mybir
import concourse.tile as tile
from concourse import bacc
from concourse import bass_utils
from concourse.masks import make_identity

T, D, H, DK, KS, PAD = 2048, 512, 4, 128, 11, 5
NCORES = 8
NT = T // 128          # 16 t-blocks of 128
NC = D // 128          # 4 d-chunks of 128
SCALE = float(DK) ** -0.5
MASK_NEG = -30000.0

F32 = mybir.dt.float32
BF16 = mybir.dt.bfloat16
AF = mybir.ActivationFunctionType
OP = mybir.AluOpType

ATT_DT = os.environ.get("SANM_ATT_DT", "bf16")   # bf16 | f32
REPS = int(os.environ.get("SANM_REPS", "1"))     # timing: repeat body in one NEFF
FSMN_DT = os.environ.get("SANM_FSMN_DT", "bf16")  # f32 | bf16


def _bcast_vec(ap, nrows):
    """Broadcast a flat [N] DRAM AP across partitions -> [nrows, N]."""
    return bass.AP(tensor=ap.tensor, offset=ap.offset, ap=[[0, nrows]] + list(ap.ap))


def _tiles(total, step=512):
    out, p = [], 0
    while p < total:
        n = min(step, total - p)
        rem = total - p - n
        if 0 < rem < 256:
            n = (n + rem) // 2
            n = (n + 127) // 128 * 128
        out.append((p, n))
        p += n
    return out


def build_kernel_body(tc, aps, TK, rep=0):
    nc = tc.nc
    (xb_d, mask_d, xcb_d, cbias_d, wqkv_d, bqkv_d, wout_d, bout_d, fw_d,
     out_d) = aps
    R = f"r{rep}_" if rep else ""
    TKC = TK // 128  # compact key chunks

    att_store = F32 if ATT_DT == "f32" else BF16

    stack = ExitStack()
    consts = stack.enter_context(tc.tile_pool(name=R + "consts", bufs=1))
    work = stack.enter_context(tc.tile_pool(name=R + "work", bufs=2))
    ps = stack.enter_context(tc.tile_pool(name=R + "ps", bufs=1, space="PSUM"))
    dram = stack.enter_context(tc.tile_pool(name=R + "dram", bufs=2, space="DRAM"))

    # p_main holds all long-lived tensors (whole kernel); p_x nests inside it
    # (LIFO) and is released after the qkv matmuls to reclaim x^T space.
    main_cm = tc.tile_pool(name=R + "p_main", bufs=1)
    x_cm = tc.tile_pool(name=R + "p_x", bufs=1)
    p_main = main_cm.__enter__()
    p_x = x_cm.__enter__()

    # ---------------- constants ----------------
    ident = consts.tile([128, 128], F32, name="ident", tag="ident")
    make_identity(nc, ident)
    ident_b = consts.tile([128, 128], att_store, name="ident_b", tag="ident_b")
    nc.vector.tensor_copy(ident_b, ident)

    ones_att = consts.tile([128, 1], att_store, name="ones_att", tag="ones_att")
    nc.vector.memset(ones_att, 1.0)

    # compact-key exp bias (0 valid / -30000 padded), as columns (128, TKC)
    mbias = consts.tile([128, TKC], F32, name="mbias", tag="mbias")
    nc.sync.dma_start(out=mbias, in_=cbias_d.rearrange("(c p) -> p c", p=128))

    # mask broadcast across partitions (128, T) bf16 (exact for 0/1), for FSMN
    mrow = consts.tile([128, T], BF16, name="mrow", tag="mrow")
    nc.gpsimd.dma_start(out=mrow, in_=_bcast_vec(mask_d, 128))

    # biases as per-partition columns
    bq = consts.tile([128, 12], F32, name="bq", tag="bq")
    nc.sync.dma_start(out=bq, in_=bqkv_d.rearrange("(c p) -> p c", p=128))
    bo = consts.tile([128, NC], F32, name="bo", tag="bo")
    nc.sync.dma_start(out=bo, in_=bout_d.rearrange("(c p) -> p c", p=128))

    # fsmn weights (128, NC, KS); center tap += 1 (folds the residual)
    wadj = consts.tile([128, NC, KS], F32, name="wadj", tag="wadj")
    nc.scalar.dma_start(out=wadj, in_=fw_d.rearrange("(c p) o k -> p c (o k)", p=128))
    if FSMN_DT == "f32":
        nc.vector.tensor_scalar_add(
            wadj[:, :, PAD : PAD + 1], wadj[:, :, PAD : PAD + 1], 1.0
        )

    # Wout (128, NC, 512) in attention dtype, loaded directly (host-cast bf16)
    wo = consts.tile([128, NC, D], att_store, name="wo", tag="wo")
    nc.scalar.dma_start(
        out=wo, in_=wout_d.rearrange("(c p) f -> p c f", p=128)
    )

    # ---------------- x^T and xc^T (PE transposes, bf16 1cyc/row) ------------
    # (DMA xbar transpose races with concurrent regular DMA traffic on real
    # hardware — it silently corrupted 7/8 cores — so transpose on PE.)
    xT = [p_x.tile([128, T], BF16, name=f"xT{c}", tag=f"xT{c}") for c in range(NC)]
    for ti in range(NT):
        xn = work.tile([128, D], BF16, name="xnat", tag="xnat", bufs=3)
        (nc.sync if ti % 2 == 0 else nc.scalar).dma_start(
            out=xn, in_=xb_d[ti * 128 : (ti + 1) * 128, :]
        )
        for c in range(NC):
            tp = ps.tile([128, 128], BF16, name="tp", tag="s", bufs=4)
            nc.tensor.transpose(tp, xn[:, c * 128 : (c + 1) * 128], ident_b)
            dst = xT[c][:, ti * 128 : (ti + 1) * 128]
            if (ti + c) % 2 == 0:
                nc.vector.tensor_copy(dst, tp)
            else:
                nc.scalar.copy(dst, tp)
    xcT = [
        p_x.tile([128, TK], BF16, name=f"xcT{c}", tag=f"xcT{c}") for c in range(NC)
    ]
    for ti in range(TKC):
        xn = work.tile([128, D], BF16, name="xnat", tag="xnat", bufs=3)
        nc.scalar.dma_start(out=xn, in_=xcb_d[ti * 128 : (ti + 1) * 128, :])
        for c in range(NC):
            tp = ps.tile([128, 128], BF16, name="tpc", tag="s", bufs=4)
            nc.tensor.transpose(tp, xn[:, c * 128 : (c + 1) * 128], ident_b)
            dst = xcT[c][:, ti * 128 : (ti + 1) * 128]
            if (ti + c) % 2 == 0:
                nc.vector.tensor_copy(dst, tp)
            else:
                nc.scalar.copy(dst, tp)

    # ---------------- qkv^T = (x @ Wqkv)^T ----------------
    # q on full tokens; k only compact; v full (FSMN) and compact (attention)
    qT = [p_main.tile([128, T], att_store, name=f"qT{h}", tag=f"qT{h}") for h in range(H)]
    kTc = [p_main.tile([128, TK], att_store, name=f"kTc{h}", tag=f"kTc{h}") for h in range(H)]
    vT = [p_main.tile([128, T], F32, name=f"vT{c}", tag=f"vT{c}") for c in range(NC)]
    vcT = [p_main.tile([128, TK], att_store, name=f"vcT{c}", tag=f"vcT{c}") for c in range(NC)]

    def project(f, src, tspans, sink):
        """psum[128, n] = Wqkv[:, f-block].T @ src over d-chunks, then sink."""
        wqf = work.tile([128, NC, 128], BF16, name="wqf", tag="wqf", bufs=3)
        wqf_src = wqkv_d[:, f * 128 : (f + 1) * 128].rearrange("(c p) f -> p c f", p=128)
        (nc.scalar if f % 2 else nc.sync).dma_start(out=wqf, in_=wqf_src)
        for t0, n in tspans:
            mm = ps.tile([128, 512], F32, name="mmq", tag="s", bufs=4)
            for dc in range(NC):
                nc.tensor.matmul(
                    mm[:, :n],
                    wqf[:, dc, :],
                    src[dc][:, t0 : t0 + n],
                    start=(dc == 0),
                    stop=(dc == NC - 1),
                )
            sink(mm, t0, n)

    for f in range(4):  # q: full tokens -> qT (bf16), bias via ACT
        def sink_q(mm, t0, n, f=f):
            nc.scalar.activation(
                qT[f][:, t0 : t0 + n], mm[:, :n], AF.Identity,
                bias=bq[:, f : f + 1], scale=1.0,
            )
        project(f, xT, _tiles(T), sink_q)
    for f in range(4, 8):  # k: compact tokens -> kTc
        def sink_k(mm, t0, n, f=f):
            nc.scalar.activation(
                kTc[f - 4][:, t0 : t0 + n], mm[:, :n], AF.Identity,
                bias=bq[:, f : f + 1], scale=1.0,
            )
        project(f, xcT, _tiles(TK), sink_k)
    for f in range(8, 12):  # v full tokens (FSMN), f32, bias via DVE
        def sink_v(mm, t0, n, f=f):
            nc.vector.tensor_scalar_add(
                vT[f - 8][:, t0 : t0 + n], mm[:, :n], bq[:, f : f + 1]
            )
        project(f, xT, _tiles(T), sink_v)
    for f in range(8, 12):  # v compact tokens (attention), att dtype
        def sink_vc(mm, t0, n, f=f):
            nc.scalar.activation(
                vcT[f - 8][:, t0 : t0 + n], mm[:, :n], AF.Identity,
                bias=bq[:, f : f + 1], scale=1.0,
            )
        project(f, xcT, _tiles(TK), sink_vc)
    x_cm.__exit__(None, None, None)  # frees xT, xcT

    # ------------- compact v natural (PE transposes of vcT, att dtype) --------
    vh = [
        p_main.tile([128, TKC, 128], att_store, name=f"vh{h}", tag=f"vh{h}")
        for h in range(H)
    ]
    for h in range(H):
        for jc in range(TKC):
            tp = ps.tile([128, 128], att_store, name="tpv", tag="s", bufs=4)
            nc.tensor.transpose(tp, vcT[h][:, jc * 128 : (jc + 1) * 128], ident_b)
            nc.scalar.copy(vh[h][:, jc, :], tp)

    # -------- FSMN (d, t layout), in place on vT; result lands back in vT -----
    for c in range(NC):
        # vm = v * m (in place)
        nc.gpsimd.tensor_tensor(vT[c], vT[c], mrow, op=OP.mult)
        if FSMN_DT == "f32":
            acc = p_main.tile([128, T], F32, name=f"facc{c}", tag="facc")
            # center tap first (w[5]+1 folds the residual), full width
            nc.any.tensor_scalar_mul(acc, vT[c], wadj[:, c, PAD : PAD + 1])
            src_t = vT[c]
        else:
            vmb = p_main.tile([128, T], BF16, name=f"vmb{c}", tag="fscratch")
            nc.any.tensor_copy(vmb, vT[c])
            acc = p_main.tile([128, T], BF16, name=f"facc{c}", tag="facc")
            nc.any.tensor_scalar_mul(acc, vmb, wadj[:, c, PAD : PAD + 1])
            src_t = vmb
        for k in list(range(0, PAD)) + list(range(PAD + 1, KS)):
            s = k - PAD
            lo, hi = max(0, -s), T - max(0, s)
            nc.vector.scalar_tensor_tensor(
                acc[:, lo:hi],
                src_t[:, lo + s : hi + s],
                wadj[:, c, k : k + 1],
                acc[:, lo:hi],
                OP.mult,
                OP.add,
            )
        if FSMN_DT == "f32":
            nc.gpsimd.tensor_tensor(acc, acc, mrow, op=OP.mult)
            # final masked conv + bias overwrites vT[c] (all tap reads are done)
            nc.any.tensor_scalar_add(vT[c], acc, bo[:, c : c + 1])
        else:
            # residual in f32: out = (conv_bf + vm) * m + bo; vm*m = vm
            facc2 = p_main.tile([128, T], F32, name=f"facc2{c}", tag="fscratch2")
            nc.vector.tensor_tensor(facc2, acc, mrow, op=OP.mult)
            nc.gpsimd.tensor_tensor(vT[c], facc2, vT[c], op=OP.add)
            nc.any.tensor_scalar_add(vT[c], vT[c], bo[:, c : c + 1])

    # ---------------- attention ----------------
    zd = dram.tile([H * T], F32, name="zd", tag="zd", bufs=1)
    ctxT = [
        p_main.tile([128, T], att_store, name=f"ctxT{h}", tag=f"ctxT{h}")
        for h in range(H)
    ]
    for h in range(H):
        for qs in range(4):  # query spans of 512
            isl = slice(qs * 512, (qs + 1) * 512)
            ctx_ps = ps.tile([128, 512], F32, name="ctx_ps", tag="actx", bufs=2)
            z_ps = ps.tile([1, 512], F32, name="z_ps", tag="z", bufs=2)
            for jc in range(TKC):
                s_ps = ps.tile([128, 512], F32, name="s_ps", tag="s", bufs=4)
                nc.tensor.matmul(
                    s_ps,
                    kTc[h][:, jc * 128 : (jc + 1) * 128],
                    qT[h][:, isl],
                    start=True,
                    stop=True,
                    skip_group_check=True,
                )
                eT = work.tile([128, 512], att_store, name="eT", tag="eT", bufs=4)
                nc.scalar.activation(
                    eT, s_ps, AF.Exp, bias=mbias[:, jc : jc + 1], scale=SCALE
                )
                nc.tensor.matmul(
                    ctx_ps,
                    vh[h][:, jc, :],
                    eT,
                    start=(jc == 0),
                    stop=(jc == TKC - 1),
                    skip_group_check=True,
                )
                nc.tensor.matmul(
                    z_ps,
                    ones_att,
                    eT,
                    start=(jc == 0),
                    stop=(jc == TKC - 1),
                    skip_group_check=True,
                )
            z_sb = work.tile([1, 512], F32, name="z_sb", tag="z_sb", bufs=2)
            nc.scalar.copy(z_sb, z_ps)
            rz = work.tile([1, 512], F32, name="rz", tag="rz", bufs=2)
            nc.vector.reciprocal(rz, z_sb)
            zsl = slice(h * T + qs * 512, h * T + (qs + 1) * 512)
            nc.sync.dma_start(out=zd[zsl], in_=rz)
            zb = work.tile([128, 512], BF16, name="zb", tag="zb", bufs=2)
            nc.gpsimd.dma_start(
                out=zb,
                in_=bass.AP(
                    tensor=zd.tensor,
                    offset=zd.offset + h * T + qs * 512,
                    ap=[[0, 128], [1, 512]],
                ),
            )
            nc.vector.tensor_tensor(ctxT[h][:, isl], ctx_ps, zb, op=OP.mult)

    # ---------------- out projection + fsmn add ----------------
    for tb in range(NT):
        op_ps = ps.tile([128, 512], F32, name="op_ps", tag="s", bufs=4)
        for h in range(H):
            nc.tensor.matmul(
                op_ps,
                ctxT[h][:, tb * 128 : (tb + 1) * 128],
                wo[:, h, :],
                start=(h == 0),
                stop=(h == H - 1),
            )
        # transpose this t-block of fsmn into natural layout
        ftp = ps.tile([128, 512], F32, name="ftp", tag="z", bufs=2)
        for c in range(NC):
            nc.tensor.transpose(
                ftp[:, c * 128 : (c + 1) * 128],
                vT[c][:, tb * 128 : (tb + 1) * 128],
                ident,
            )
        f_sb = work.tile([128, D], F32, name="f_sb", tag="f_sb", bufs=2)
        nc.scalar.copy(f_sb, ftp)
        o_sb = work.tile([128, D], F32, name="o_sb", tag="o_sb", bufs=2)
        nc.vector.tensor_tensor(o_sb, op_ps, f_sb, op=OP.add)
        nc.sync.dma_start(out=out_d[tb * 128 : (tb + 1) * 128, :], in_=o_sb)

    main_cm.__exit__(None, None, None)
    stack.close()


def _cached_spmd_fn(nc, donate=True):
    """Build (once) and cache a jitted SPMD executor for this nc.

    run_bass_kernel_spmd -> run_bass_via_pjrt re-creates the jax.jit closure
    on every call, so every kernel() invocation pays a full retrace + BIR
    serialization (scales with instruction count / REPS). Caching the jitted
    callable makes repeat calls pure dispatch+transfer+execute.

    donate=False builds a variant safe to call repeatedly on device-resident
    input buffers (for timing): the kernel writes every element of `out`, so
    uninitialized result buffers are fine.
    """
    cache = getattr(nc, "_sanm_exec", None)
    if cache is None:
        cache = nc._sanm_exec = {}
    ent = cache.get(donate)
    if ent is not None:
        return ent
    import jax
    from jax.experimental.shard_map import shard_map
    from jax.sharding import Mesh, PartitionSpec

    from concourse import bass2jax

    bass2jax.install_neuronx_cc_hook()
    assert nc.dbg_addr is None
    partition_name = nc.partition_id_tensor.name if nc.partition_id_tensor else None

    in_names, out_names, out_avals, zero_shapes = [], [], [], []
    for alloc in nc.m.functions[0].allocations:
        if not isinstance(alloc, mybir.MemoryLocationSet):
            continue
        name = alloc.memorylocations[0].name
        if alloc.kind == "ExternalInput":
            if name != partition_name:
                in_names.append(name)
        elif alloc.kind == "ExternalOutput":
            shape = tuple(alloc.tensor_shape)
            dtype = mybir.dt.np(alloc.dtype)
            out_avals.append(jax.core.ShapedArray(shape, dtype))
            out_names.append(name)
            zero_shapes.append((shape, dtype))
    n_params = len(in_names)
    n_outs = len(out_avals)
    all_names = list(in_names) + list(out_names)
    if partition_name is not None:
        all_names.append(partition_name)
    donate_idx = tuple(range(n_params, n_params + n_outs))

    def _body(*args):
        operands = list(args)
        if partition_name is not None:
            operands.append(bass2jax.partition_id_tensor())
        outs = bass2jax._bass_exec_p.bind(
            *operands,
            out_avals=tuple(out_avals),
            in_names=tuple(all_names),
            out_names=tuple(out_names),
            lowering_input_output_aliases=(),
            sim_require_finite=True,
            sim_require_nnan=True,
            nc=nc,
        )
        return tuple(outs)

    devices = jax.devices()[:NCORES]
    mesh = Mesh(np.asarray(devices), ("core",))
    sharded = jax.jit(
        shard_map(
            _body,
            mesh=mesh,
            in_specs=(PartitionSpec("core"),) * (n_params + n_outs),
            out_specs=(PartitionSpec("core"),) * n_outs,
            check_rep=False,
        ),
        donate_argnums=donate_idx if donate else (),
        keep_unused=True,
    )
    ent = (sharded, in_names, out_names, out_avals, zero_shapes, n_params, mesh)
    cache[donate] = ent
    return ent


def _run_cached(nc, in_maps):
    sharded, in_names, out_names, out_avals, zero_shapes, n_params, _ = _cached_spmd_fn(nc)
    concat_in = [
        np.concatenate([np.asarray(m[name]) for m in in_maps], axis=0)
        for name in in_names
    ]
    concat_zeros = [
        np.zeros((NCORES * s[0], *s[1:]), dt) for (s, dt) in zero_shapes
    ]
    out_arrs = sharded(*concat_in, *concat_zeros)
    return [
        {
            name: np.asarray(out_arrs[i]).reshape(NCORES, *out_avals[i].shape)[c]
            for i, name in enumerate(out_names)
        }
        for c in range(NCORES)
    ]


_CACHE = {}


def _build(TK):
    key = (ATT_DT, REPS, TK, FSMN_DT)
    if key in _CACHE:
        return _CACHE[key]
    nc = bacc.Bacc(
        "TRN2",
        target_bir_lowering=False,
        debug=False,
        enable_asserts=False,
        num_devices=NCORES,
    )
    aps = (
        nc.dram_tensor("xb", (T, D), BF16, kind="ExternalInput").ap(),
        nc.dram_tensor("mask", (T,), F32, kind="ExternalInput").ap(),
        nc.dram_tensor("xcb", (TK, D), BF16, kind="ExternalInput").ap(),
        nc.dram_tensor("cbias", (TK,), F32, kind="ExternalInput").ap(),
        nc.dram_tensor("Wqkv", (D, 3 * D), BF16, kind="ExternalInput").ap(),
        nc.dram_tensor("bqkv", (3 * D,), F32, kind="ExternalInput").ap(),
        nc.dram_tensor("Wout", (D, D), BF16, kind="ExternalInput").ap(),
        nc.dram_tensor("bout", (D,), F32, kind="ExternalInput").ap(),
        nc.dram_tensor("fsmn_w", (D, 1, KS), F32, kind="ExternalInput").ap(),
        nc.dram_tensor("out", (T, D), F32, kind="ExternalOutput").ap(),
    )
    with tile.TileContext(nc) as tc:
        for rep in range(REPS):
            build_kernel_body(tc, aps, TK, rep)
    nc.compile()
    _CACHE[key] = nc
    return nc


def _bf16():
    import ml_dtypes

    return ml_dtypes.bfloat16


def _compact(x_b, mask_b, TK):
    """Host-side gather of unmasked token rows, padded to TK (bf16)."""
    idx = np.nonzero(mask_b != 0)[0]
    n = len(idx)
    xc = np.zeros((TK, x_b.shape[1]), _bf16())
    xc[:n] = x_b[idx[:TK]].astype(_bf16())
    cb = np.full((TK,), MASK_NEG, np.float32)
    cb[:n] = 0.0
    return xc, cb


def _prep(x, mask, Wqkv, bqkv, Wout, bout, fsmn_w):
    """Full inputs -> (TK, per-core in_maps) with host-side bf16 casts."""
    bf16 = _bf16()
    x = np.ascontiguousarray(np.asarray(x, dtype=np.float32))
    mask = np.ascontiguousarray(np.asarray(mask, dtype=np.float32))
    Wqkv_b = np.ascontiguousarray(np.asarray(Wqkv, dtype=np.float32).astype(bf16))
    bqkv = np.ascontiguousarray(np.asarray(bqkv, dtype=np.float32))
    Wout_b = np.ascontiguousarray(np.asarray(Wout, dtype=np.float32).astype(bf16))
    bout = np.ascontiguousarray(np.asarray(bout, dtype=np.float32))
    fsmn_w = np.ascontiguousarray(np.asarray(fsmn_w, dtype=np.float32))

    counts = [int((mask[b, 0] != 0).sum()) for b in range(NCORES)]
    TK = min(T, max(256, int(-(-max(counts) // 128) * 128)))
    in_maps = []
    for b in range(NCORES):
        xc, cb = _compact(x[b], mask[b, 0], TK)
        in_maps.append(
            {
                "xb": np.ascontiguousarray(x[b].astype(bf16)),
                "mask": np.ascontiguousarray(mask[b, 0]),
                "xcb": xc,
                "cbias": cb,
                "Wqkv": Wqkv_b,
                "bqkv": bqkv,
                "Wout": Wout_b,
                "bout": bout,
                "fsmn_w": fsmn_w,
            }
        )
    return TK, in_maps


def kernel(x, mask, Wqkv, bqkv, Wout, bout, fsmn_w):
    TK, in_maps = _prep(x, mask, Wqkv, bqkv, Wout, bout, fsmn_w)
    nc = _build(TK)
    results = _run_cached(nc, in_maps)
    out = np.stack([results[b]["out"] for b in range(NCORES)], axis=0)
    return out


if __name__ == "__main__":
    rng = np.random.default_rng(0)
    ins = {
        "x": rng.standard_normal((NCORES, T, D), dtype=np.float32),
        "mask": rng.integers(0, 2, (NCORES, 1, T)).astype(np.float32),
        "Wqkv": (rng.standard_normal((D, 3 * D)) * 0.02).astype(np.float32),
        "bqkv": np.zeros((3 * D,), np.float32),
        "Wout": (rng.standard_normal((D, D)) * 0.02).astype(np.float32),
        "bout": np.zeros((D,), np.float32),
        "fsmn_w": (rng.standard_normal((D, 1, KS)) * 0.1).astype(np.float32),
    }
    out = kernel(**ins)
    print(out.shape, out.dtype, float(np.abs(out).max()))
